# revision 4
# baseline (speedup 1.0000x reference)
"""KMaxPooling (top-8 along seq axis) Bass kernel for TRN2, 8-core SPMD.

Input  x: (64, 4096, 256) fp32. Output: (64, 8, 256) fp32 = per (batch,
channel) the 8 largest values over the 4096 seq positions, descending.

Strategy (per core, batch-sharded 8 ways -> 8 batches/core, 32 MB):
  - contiguous loads: partition p holds rows r0 + p*T .. r0 + p*T + T-1
    of a row range, so every partition line is one big (4-16 KB) DMA
    descriptor and the whole load reads HBM sequentially. Top-8 over seq
    is permutation invariant, so the block shuffle is harmless.
  - ALL loads ride ONE HWDGE queue: the 16 DMA engines are a shared
    pool, so concurrent queues only delay first-completion. FIFO on one
    queue keeps completions in issue order at full (~470 GB/s) rate.
  - load sizes: first batch in 1 MB quarters (prime the pipeline fast),
    middle batches in 2 MB halves, last batch 2M/1M/.5M/.5M (short tail).
  - PE transposes 128x128 blocks into PSUM so channels land on partitions
  - DVE InstMax (hardware top-8, sorted desc) over PSUM spans; DVE is the
    critical engine (~1 elem/cycle @ 0.96 GHz -> ~68 us/core minimum)
  - tiny second-level InstMax merges the per-span candidates
  - one 64 KB output DMA per core; host reassembles pure layout
"""

import sys

sys.path.insert(0, "/opt/trn_rl_repo")

import numpy as np

import concourse.bass as bass
import concourse.mybir as mybir
from concourse import masks
from concourse.tile import TileContext
from concourse.vector_clock import ScopedClock, VectorClock
from concourse.bass_utils import run_bass_kernel_spmd

B, S, C, K = 64, 4096, 256, 8
NCORES = 8
BPC = B // NCORES  # batches per core
CH_GROUPS = C // 128  # 2

F32 = mybir.dt.float32

N_PROCS = 27


class SplitDrainTileContext(TileContext):
    """The walrus backend here rejects any instruction carrying more than
    one sync wait ("Too many sync wait commands"), but Tile's semaphore
    assignment can attach several. Two fixes:

    1. _lower_ordered_insts: before lowering, hoist excess waits of every
       scheduled instruction onto single-wait same-engine NoOps inserted
       right before it.
    2. _drain_and_barrier: emit one single-wait drain per logical proc
       instead of one drain waiting on the whole global vector clock.
    """

    def _lower_ordered_insts(self, ordered):
        for bb_name, insts in ordered.items():
            rewritten = []
            for inst in insts:
                si = inst.sync_info
                if si is not None and si.on_wait and len(si.on_wait) > 1:
                    waits = list(si.on_wait)
                    for k, w in enumerate(waits[:-1]):
                        nop = mybir.InstNoOp(
                            name=f"{inst.name}.wsplit{k}",
                            engine=inst.engine,
                            sync_info=mybir.SyncInfo(on_wait=[w], on_update=[]),
                            bass_nofuse=True,
                        )
                        rewritten.append(nop)
                    si.on_wait = waits[-1:]
                rewritten.append(inst)
            ordered[bb_name] = rewritten
        return super()._lower_ordered_insts(ordered)

    def _drain_and_barrier(self, tick_clock, wait_clock):
        gc = tick_clock.global_clock
        for p in range(N_PROCS):
            if gc[p] > 0:
                v = [0] * N_PROCS
                v[p] = gc[p]
                di = self.nc.sync.drain()
                wait_clock.add_sem_waits(di.ins, ScopedClock({None: VectorClock(v)}))

        self.nc.all_engine_barrier()
        assert self.sems is not None
        popped = self.nc._tile_sem_poison_stack.pop()
        assert popped is self._sem_poison
        self.nc.clear_and_free_semaphores(list(self.sems.allocated().values()))
        self.nc.all_engine_barrier()


def build_program():
    nc = bass.Bass()
    x_ext = nc.declare_dram_parameter("x", [BPC, S, C], F32, isOutput=False)
    # out[c', g*64 + b*8 + k]: top-k values of channel g*128+c' in batch b
    out_ext = nc.declare_dram_parameter(
        "out", [128, CH_GROUPS * BPC * K], F32, isOutput=True
    )

    with SplitDrainTileContext(nc) as tc:
        with (
            tc.tile_pool(name="const", bufs=1) as const_pool,
            tc.tile_pool(name="xin", bufs=8) as in_pool,
            tc.tile_pool(name="psum", bufs=2, space="PSUM") as psum_pool,
            tc.tile_pool(name="cand", bufs=4) as cand_pool,
            tc.tile_pool(name="obuf", bufs=1) as out_pool,
        ):
            identity = const_pool.tile([128, 128], F32)
            masks.make_identity(nc, identity[:])

            obuf = out_pool.tile([128, CH_GROUPS * BPC * K], F32)

            def load_rows(b, r0, r1):
                """One contiguous DMA of x[b, r0:r1] with partition p
                holding rows r0 + p*T .. r0 + p*T + T-1 (T KB descriptor
                per partition). All loads ride the sync-engine queue so
                completions arrive FIFO. Returns (xin, T)."""
                nrows = r1 - r0
                T = nrows // 128
                xin = in_pool.tile([128, T * C], F32, name="xin", tag="xin")
                nc.sync.dma_start(
                    out=xin[:],
                    in_=x_ext[b, r0:r1].rearrange("(p t) c -> p (t c)", p=128),
                )
                return xin, T

            def transpose_span(xin, g, ps, s0, t0, nt):
                """Transpose blocks t0..t0+nt-1 of group g from xin into
                ps columns starting at slot s0 (128 cols per slot)."""
                for i in range(nt):
                    col = (t0 + i) * C + g * 128
                    s = s0 + i
                    nc.tensor.matmul(
                        ps[:, 128 * s : 128 * (s + 1)],
                        xin[:, col : col + 128],
                        identity[:],
                        is_transpose=True,
                        start=True,
                        stop=True,
                    )

            def process_spans(xin, spans, cands, slot):
                """For each channel group, transpose `xin` into a fresh
                PSUM tile and top-8 it into cands[g][:, slot*K:...]."""
                for g in range(CH_GROUPS):
                    ps = psum_pool.tile(
                        [128, 128 * spans], F32, name="ps", tag="ps"
                    )
                    transpose_span(xin, g, ps, 0, 0, spans)
                    nc.vector.max(
                        out=cands[g][:, slot * K : (slot + 1) * K], in_=ps[:]
                    )

            def merge(cands, b):
                for g in range(CH_GROUPS):
                    nc.vector.max(
                        out=obuf[:, (g * BPC + b) * K : (g * BPC + b + 1) * K],
                        in_=cands[g][:],
                    )

            # batch 0: 512K/512K/1M/2M loads -> the first MAX8 fires as
            # early as possible and load sizes ramp up as the pipe fills.
            cands = [
                cand_pool.tile([128, 4 * K], F32, name="cand", tag="cand")
                for _ in range(CH_GROUPS)
            ]
            b0_cuts = [0, S // 8, S // 4, S // 2, S]
            for q in range(4):
                xin, T = load_rows(0, b0_cuts[q], b0_cuts[q + 1])
                process_spans(xin, T, cands, q)
            merge(cands, 0)

            # batches 1..BPC-2: 2 MB half loads -> 2048-wide spans
            for b in range(1, BPC - 1):
                cands = [
                    cand_pool.tile([128, 2 * K], F32, name="cand", tag="cand")
                    for _ in range(CH_GROUPS)
                ]
                for h in range(2):
                    xin, T = load_rows(b, h * S // 2, (h + 1) * S // 2)
                    process_spans(xin, 16, cands, h)
                merge(cands, b)

            # last batch: 2MB + 1MB + 512KB + 512KB loads so the tail after
            # the final DMA packet is short.
            b = BPC - 1
            cands = [
                cand_pool.tile([128, 4 * K], F32, name="cand", tag="cand")
                for _ in range(CH_GROUPS)
            ]
            xin, T = load_rows(b, 0, S // 2)
            process_spans(xin, 16, cands, 0)
            xin, T = load_rows(b, S // 2, 3 * S // 4)
            process_spans(xin, 8, cands, 1)
            for e in range(2):
                xin, T = load_rows(b, (6 + e) * S // 8, (7 + e) * S // 8)
                process_spans(xin, 4, cands, 2 + e)
            merge(cands, b)

            nc.sync.dma_start(out=out_ext[:], in_=obuf[:])

    return nc


_prog = None


def _get_prog():
    global _prog
    if _prog is None:
        _prog = build_program()
    return _prog


def run_on_cores(x: np.ndarray, **run_kwargs):
    """Shard, run on 8 cores, return (full_output, BassKernelResults)."""
    nc = _get_prog()
    x = np.ascontiguousarray(np.asarray(x, dtype=np.float32))
    in_maps = [
        {"x": np.ascontiguousarray(x[i * BPC : (i + 1) * BPC])} for i in range(NCORES)
    ]
    res = run_bass_kernel_spmd(nc, in_maps, list(range(NCORES)), **run_kwargs)
    parts = []
    for i in range(NCORES):
        o = res.results[i]["out"]  # (128, CH_GROUPS*BPC*K)
        o = o.reshape(128, CH_GROUPS, BPC, K)  # (c', g, b, k)
        o = o.transpose(2, 3, 1, 0).reshape(BPC, K, C)  # (b, k, g*128+c')
        parts.append(o)
    return np.concatenate(parts, axis=0), res


def kernel(x: np.ndarray) -> np.ndarray:
    out, _ = run_on_cores(x)
    return out


# revision 5
# speedup vs baseline: 1.0340x; 1.0340x over previous
"""KMaxPooling (top-8 along seq axis) Bass kernel for TRN2, 8-core SPMD.

Input  x: (64, 4096, 256) fp32. Output: (64, 8, 256) fp32 = per (batch,
channel) the 8 largest values over the 4096 seq positions, descending.

Strategy (per core, batch-sharded 8 ways -> 8 batches/core, 32 MB):
  - contiguous loads: partition p holds rows r0 + p*T .. r0 + p*T + T-1
    of a row range, so every partition line is one big (4-16 KB) DMA
    descriptor and the whole load reads HBM sequentially. Top-8 over seq
    is permutation invariant, so the block shuffle is harmless.
  - ALL loads ride ONE HWDGE queue: the 16 DMA engines are a shared
    pool, so concurrent queues only delay first-completion. FIFO on one
    queue keeps completions in issue order at full (~470 GB/s) rate.
  - load sizes: first batch in 1 MB quarters (prime the pipeline fast),
    middle batches in 2 MB halves, last batch 2M/1M/.5M/.5M (short tail).
  - PE transposes 128x128 blocks into PSUM so channels land on partitions
  - DVE InstMax (hardware top-8, sorted desc) over PSUM spans; DVE is the
    critical engine (~1 elem/cycle @ 0.96 GHz -> ~68 us/core minimum)
  - tiny second-level InstMax merges the per-span candidates
  - one 64 KB output DMA per core; host reassembles pure layout
"""

import sys

sys.path.insert(0, "/opt/trn_rl_repo")

import numpy as np

import concourse.bass as bass
import concourse.mybir as mybir
from concourse import masks
from concourse.tile import TileContext
from concourse.vector_clock import ScopedClock, VectorClock
from concourse.bass_utils import run_bass_kernel_spmd

B, S, C, K = 64, 4096, 256, 8
NCORES = 8
BPC = B // NCORES  # batches per core
CH_GROUPS = C // 128  # 2

F32 = mybir.dt.float32

N_PROCS = 27


class SplitDrainTileContext(TileContext):
    """The walrus backend here rejects any instruction carrying more than
    one sync wait ("Too many sync wait commands"), but Tile's semaphore
    assignment can attach several. Two fixes:

    1. _lower_ordered_insts: before lowering, hoist excess waits of every
       scheduled instruction onto single-wait same-engine NoOps inserted
       right before it.
    2. _drain_and_barrier: emit one single-wait drain per logical proc
       instead of one drain waiting on the whole global vector clock.
    """

    def _lower_ordered_insts(self, ordered):
        for bb_name, insts in ordered.items():
            rewritten = []
            for inst in insts:
                si = inst.sync_info
                if si is not None and si.on_wait and len(si.on_wait) > 1:
                    waits = list(si.on_wait)
                    for k, w in enumerate(waits[:-1]):
                        nop = mybir.InstNoOp(
                            name=f"{inst.name}.wsplit{k}",
                            engine=inst.engine,
                            sync_info=mybir.SyncInfo(on_wait=[w], on_update=[]),
                            bass_nofuse=True,
                        )
                        rewritten.append(nop)
                    si.on_wait = waits[-1:]
                rewritten.append(inst)
            ordered[bb_name] = rewritten
        return super()._lower_ordered_insts(ordered)

    def _drain_and_barrier(self, tick_clock, wait_clock):
        gc = tick_clock.global_clock
        for p in range(N_PROCS):
            if gc[p] > 0:
                v = [0] * N_PROCS
                v[p] = gc[p]
                di = self.nc.sync.drain()
                wait_clock.add_sem_waits(di.ins, ScopedClock({None: VectorClock(v)}))

        self.nc.all_engine_barrier()
        assert self.sems is not None
        popped = self.nc._tile_sem_poison_stack.pop()
        assert popped is self._sem_poison
        self.nc.clear_and_free_semaphores(list(self.sems.allocated().values()))
        self.nc.all_engine_barrier()


def build_program():
    nc = bass.Bass()
    x_ext = nc.declare_dram_parameter("x", [BPC, S, C], F32, isOutput=False)
    # out[c', g*64 + b*8 + k]: top-k values of channel g*128+c' in batch b
    out_ext = nc.declare_dram_parameter(
        "out", [128, CH_GROUPS * BPC * K], F32, isOutput=True
    )

    with SplitDrainTileContext(nc) as tc:
        with (
            tc.tile_pool(name="const", bufs=1) as const_pool,
            tc.tile_pool(name="xin", bufs=8) as in_pool,
            tc.tile_pool(name="psum", bufs=2, space="PSUM") as psum_pool,
            tc.tile_pool(name="cand", bufs=4) as cand_pool,
            tc.tile_pool(name="obuf", bufs=1) as out_pool,
        ):
            identity = const_pool.tile([128, 128], F32)
            masks.make_identity(nc, identity[:])

            obuf = out_pool.tile([128, CH_GROUPS * BPC * K], F32)

            def load_rows(b, r0, r1):
                """One contiguous DMA of x[b, r0:r1] with partition p
                holding rows r0 + p*T .. r0 + p*T + T-1 (T KB descriptor
                per partition). All loads ride the sync-engine queue so
                completions arrive FIFO. Returns (xin, T)."""
                nrows = r1 - r0
                T = nrows // 128
                xin = in_pool.tile([128, T * C], F32, name="xin", tag="xin")
                nc.sync.dma_start(
                    out=xin[:],
                    in_=x_ext[b, r0:r1].rearrange("(p t) c -> p (t c)", p=128),
                )
                return xin, T

            def transpose_span(xin, g, ps, s0, t0, nt):
                """Transpose blocks t0..t0+nt-1 of group g from xin into
                ps columns starting at slot s0 (128 cols per slot)."""
                for i in range(nt):
                    col = (t0 + i) * C + g * 128
                    s = s0 + i
                    nc.tensor.matmul(
                        ps[:, 128 * s : 128 * (s + 1)],
                        xin[:, col : col + 128],
                        identity[:],
                        is_transpose=True,
                        start=True,
                        stop=True,
                    )

            def process_spans(xin, spans, cands, slot):
                """For each channel group, transpose `xin` into a fresh
                PSUM tile and top-8 it into cands[g][:, slot*K:...].
                PSUM tiles are always allocated 2048 wide (4 banks) so
                every allocation lands on the same bank alignment; small
                spans just use a prefix."""
                for g in range(CH_GROUPS):
                    ps = psum_pool.tile([128, 2048], F32, name="ps", tag="ps")
                    transpose_span(xin, g, ps, 0, 0, spans)
                    nc.vector.max(
                        out=cands[g][:, slot * K : (slot + 1) * K],
                        in_=ps[:, : 128 * spans],
                    )

            def merge(cands, b):
                for g in range(CH_GROUPS):
                    nc.vector.max(
                        out=obuf[:, (g * BPC + b) * K : (g * BPC + b + 1) * K],
                        in_=cands[g][:],
                    )

            # batch 0: 512K/512K/1M/2M loads -> the first MAX8 fires as
            # early as possible and load sizes ramp up as the pipe fills.
            cands = [
                cand_pool.tile([128, 4 * K], F32, name="cand", tag="cand")
                for _ in range(CH_GROUPS)
            ]
            b0_cuts = [0, S // 8, S // 4, S // 2, S]
            for q in range(4):
                xin, T = load_rows(0, b0_cuts[q], b0_cuts[q + 1])
                process_spans(xin, T, cands, q)
            merge(cands, 0)

            # batches 1..BPC-2: 2 MB half loads -> 2048-wide spans
            for b in range(1, BPC - 1):
                cands = [
                    cand_pool.tile([128, 2 * K], F32, name="cand", tag="cand")
                    for _ in range(CH_GROUPS)
                ]
                for h in range(2):
                    xin, T = load_rows(b, h * S // 2, (h + 1) * S // 2)
                    process_spans(xin, 16, cands, h)
                merge(cands, b)

            # last batch: 2MB + 1MB + 512KB + 512KB loads so the tail after
            # the final DMA packet is short.
            b = BPC - 1
            cands = [
                cand_pool.tile([128, 4 * K], F32, name="cand", tag="cand")
                for _ in range(CH_GROUPS)
            ]
            xin, T = load_rows(b, 0, S // 2)
            process_spans(xin, 16, cands, 0)
            xin, T = load_rows(b, S // 2, 3 * S // 4)
            process_spans(xin, 8, cands, 1)
            for e in range(2):
                xin, T = load_rows(b, (6 + e) * S // 8, (7 + e) * S // 8)
                process_spans(xin, 4, cands, 2 + e)
            merge(cands, b)

            nc.sync.dma_start(out=out_ext[:], in_=obuf[:])

    return nc


_prog = None


def _get_prog():
    global _prog
    if _prog is None:
        _prog = build_program()
    return _prog


def run_on_cores(x: np.ndarray, **run_kwargs):
    """Shard, run on 8 cores, return (full_output, BassKernelResults)."""
    nc = _get_prog()
    x = np.ascontiguousarray(np.asarray(x, dtype=np.float32))
    in_maps = [
        {"x": np.ascontiguousarray(x[i * BPC : (i + 1) * BPC])} for i in range(NCORES)
    ]
    res = run_bass_kernel_spmd(nc, in_maps, list(range(NCORES)), **run_kwargs)
    parts = []
    for i in range(NCORES):
        o = res.results[i]["out"]  # (128, CH_GROUPS*BPC*K)
        o = o.reshape(128, CH_GROUPS, BPC, K)  # (c', g, b, k)
        o = o.transpose(2, 3, 1, 0).reshape(BPC, K, C)  # (b, k, g*128+c')
        parts.append(o)
    return np.concatenate(parts, axis=0), res


def kernel(x: np.ndarray) -> np.ndarray:
    out, _ = run_on_cores(x)
    return out
